# revision 14
# baseline (speedup 1.0000x reference)
"""Trainium2 Bass kernel for CombinedAttention+LN (b=2, n=2048, dim=512, h=8).

Sharding: head-parallel across 8 cores (core c owns head c for both batches).
Per core: QKV projection for its head (tensor-parallel W_in column slice),
q/k layernorm, attention with host-precomputed positional bias, out-proj
partial product (W_out row slice). Host sums the 8 partials (after per-row
softmax-denominator scaling, which commutes with the out-projection) and
adds b_out.
"""

import sys

sys.path.insert(0, "/opt/trn_rl_repo")

import numpy as np

import concourse.bass as bass
import concourse.mybir as mybir
import concourse.tile as tile
from concourse import bacc
from concourse import bass_utils
from concourse.masks import make_identity

F32 = mybir.dt.float32
F32R = mybir.dt.float32r
AF = mybir.ActivationFunctionType
OP = mybir.AluOpType

B, N, DIM, H = 2, 2048, 512, 8
D = DIM // H          # 64
T = B * N             # 4096 tokens total (both batches)
NCHUNK = T // 128     # 32 token chunks of 128
JT = N // 128         # 16 key tiles per pair
EPS = 1e-5

_COMPILED = None


def _build(dump=False):
    nc = bacc.Bacc("TRN2", target_bir_lowering=False, debug=False, num_devices=8)

    x_d = nc.dram_tensor("x", [T, DIM], F32, kind="ExternalInput")
    wqkv_d = nc.dram_tensor("wqkv", [DIM, 3 * D], F32, kind="ExternalInput")
    bqkv_d = nc.dram_tensor("bqkv", [1, 3 * D], F32, kind="ExternalInput")
    lnp_d = nc.dram_tensor("lnp", [D, 4], F32, kind="ExternalInput")
    wout_d = nc.dram_tensor("wout", [D, DIM], F32R, kind="ExternalInput")
    bias_d = nc.dram_tensor("bias", [N, N], F32, kind="ExternalInput")
    y_d = nc.dram_tensor("y", [T, DIM], F32, kind="ExternalOutput")
    s_d = nc.dram_tensor("s", [1, T], F32R, kind="ExternalOutput")

    with tile.TileContext(nc) as tc:
        with tc.tile_pool(name="const", bufs=1) as const, \
             tc.tile_pool(name="big", bufs=1) as big, \
             tc.tile_pool(name="work", bufs=3) as work, \
             tc.tile_pool(name="ln", bufs=4) as lnp_pool, \
             tc.tile_pool(name="psA", bufs=2, space="PSUM") as psA:

            # ---- constants ----
            wqkv_sb = const.tile([128, 4, 3 * D], F32, tag="wqkv")
            nc.sync.dma_start(
                out=wqkv_sb, in_=wqkv_d.ap().rearrange("(s p) f -> p s f", p=128)
            )
            bqkv_sb = const.tile([1, 3 * D], F32, tag="bqkv")
            nc.sync.dma_start(out=bqkv_sb, in_=bqkv_d.ap())
            lnp_sb = const.tile([D, 4], F32, tag="lnp")
            nc.sync.dma_start(out=lnp_sb, in_=lnp_d.ap())
            wout_sb = const.tile([D, DIM], F32R, tag="wout")
            nc.sync.dma_start(out=wout_sb, in_=wout_d.ap())
            ident = const.tile([128, 128], F32, tag="ident")
            make_identity(nc, ident)
            ones1 = const.tile([1, 128], F32, tag="ones1")
            nc.vector.memset(ones1, 1.0)
            eps_t = const.tile([128, 1], F32, tag="eps")
            nc.vector.memset(eps_t, EPS)

            # ---- persistent buffers ----
            qT = big.tile([D, T], F32R, tag="qT")        # [64, 4096]
            kT = big.tile([D, T], F32R, tag="kT")
            vext = big.tile([128, B, JT, D + 1], F32R, tag="vext")
            oT = big.tile([D + 1, T], F32R, tag="oT")     # rows 0..63 attnT, row 64 sums

            # ================= Phase 1: QKV + LN + transposes =================
            with tc.tile_pool(name="psB", bufs=4, space="PSUM") as psB:
                for c in range(NCHUNK):
                    pair, j = c // JT, c % JT
                    xc = work.tile([128, DIM], F32, tag="xc")
                    nc.sync.dma_start(out=xc, in_=x_d.ap()[c * 128:(c + 1) * 128, :])

                    xTc = work.tile([128, 4, 128], F32, tag="xTc")
                    for s4 in range(4):
                        xt_ps = psB.tile([128, 128], F32, tag="tr")
                        nc.tensor.transpose(xt_ps, xc[:, s4 * 128:(s4 + 1) * 128], ident)
                        if s4 < 2:
                            nc.scalar.copy(out=xTc[:, s4, :], in_=xt_ps)
                        else:
                            nc.vector.tensor_copy(out=xTc[:, s4, :], in_=xt_ps)

                    qkv_ps = psA.tile([128, 3 * D], F32, tag="sc")
                    for s4 in range(4):
                        nc.tensor.matmul(qkv_ps, xTc[:, s4, :], wqkv_sb[:, s4, :],
                                         start=(s4 == 0), stop=False)
                    nc.tensor.matmul(qkv_ps, ones1, bqkv_sb, start=False, stop=True)

                    qkv_sb = work.tile([128, 3 * D], F32, tag="qkv")
                    nc.scalar.copy(out=qkv_sb, in_=qkv_ps)

                    # v (+ implicit ones col already set)
                    nc.gpsimd.tensor_copy(out=vext[:, pair, j, 0:D],
                                          in_=qkv_sb[:, 2 * D:3 * D])
                    nc.gpsimd.memset(vext[:, pair, j, D:D + 1].bitcast(F32), 1.0)

                    # LN stats for q and k
                    st = lnp_pool.tile([128, 2, 6], F32, tag="st")
                    nc.vector.bn_stats(out=st[:, 0, :], in_=qkv_sb[:, 0:D])
                    nc.vector.bn_stats(out=st[:, 1, :], in_=qkv_sb[:, D:2 * D])
                    mv = lnp_pool.tile([128, 2, 2], F32, tag="mv")
                    nc.vector.bn_aggr(out=mv[:, 0, :], in_=st[:, 0, :])
                    nc.vector.bn_aggr(out=mv[:, 1, :], in_=st[:, 1, :])
                    # rstd = 1/sqrt(var + eps) for both q,k in one go (strided var view)
                    sd = lnp_pool.tile([128, 2], F32, tag="sd")
                    nc.scalar.activation(out=sd, in_=mv[:, :, 1], func=AF.Sqrt,
                                         bias=eps_t, scale=1.0)
                    r2 = lnp_pool.tile([128, 2], F32, tag="r2")
                    nc.vector.reciprocal(out=r2, in_=sd)
                    nmr = lnp_pool.tile([128, 2], F32, tag="nmr")
                    nc.vector.tensor_tensor(out=nmr, in0=mv[:, :, 0], in1=r2,
                                            op=OP.mult)
                    nc.vector.tensor_scalar_mul(nmr, nmr, -1.0)

                    # apply LN (standardize) then transpose, then w*x+b into qT/kT
                    qs = work.tile([128, 2, D], F32, tag="qs")
                    for qi in range(2):
                        nc.vector.tensor_scalar(
                            out=qs[:, qi, :], in0=qkv_sb[:, qi * D:(qi + 1) * D],
                            scalar1=r2[:, qi:qi + 1], scalar2=nmr[:, qi:qi + 1],
                            op0=OP.mult, op1=OP.add)
                    for qi, dst in ((0, qT), (1, kT)):
                        t_ps = psB.tile([D, 128], F32, tag="tr")
                        nc.tensor.transpose(t_ps, qs[:, qi, :], ident)
                        nc.vector.tensor_scalar(
                            out=dst[:, c * 128:(c + 1) * 128], in0=t_ps,
                            scalar1=lnp_sb[:, 2 * qi:2 * qi + 1],
                            scalar2=lnp_sb[:, 2 * qi + 1:2 * qi + 2],
                            op0=OP.mult, op1=OP.add)

            # ================= Phase 2: attention =================
            with tc.tile_pool(name="psC", bufs=2, space="PSUM") as psC, \
                 tc.tile_pool(name="bpool", bufs=3) as bpool, \
                 tc.tile_pool(name="epool", bufs=4) as epool:
                for ih in range(2):
                    oT_ps = [psC.tile([D + 1, 1024], F32, tag="oT",
                                      name=f"oT_ps_{ih}_{p_}") for p_ in range(B)]
                    for j in range(JT):
                        bt = bpool.tile([128, 1024], F32, tag="bt")
                        nc.sync.dma_start(
                            out=bt,
                            in_=bias_d.ap()[j * 128:(j + 1) * 128,
                                            ih * 1024:(ih + 1) * 1024])
                        for pair in range(B):
                            s_ps = psA.tile([128, 1024], F32, tag="sc")
                            for nh in range(2):
                                nc.tensor.matmul(
                                    s_ps[:, nh * 512:(nh + 1) * 512],
                                    kT[:, pair * N + j * 128:pair * N + (j + 1) * 128],
                                    qT[:, pair * N + ih * 1024 + nh * 512:
                                       pair * N + ih * 1024 + (nh + 1) * 512],
                                    start=True, stop=True)
                            e0_sb = epool.tile([128, 1024], F32, tag="sb")
                            nc.scalar.activation(out=e0_sb, in_=s_ps, func=AF.Exp)
                            e_sb = epool.tile([128, 1024], F32R, tag="e")
                            eng = nc.gpsimd if (j * B + pair) % 3 == 2 else nc.vector
                            eng.tensor_tensor(out=e_sb, in0=e0_sb, in1=bt, op=OP.mult)
                            for nh in range(2):
                                nc.tensor.matmul(
                                    oT_ps[pair][:, nh * 512:(nh + 1) * 512],
                                    vext[:, pair, j, :],
                                    e_sb[:, nh * 512:(nh + 1) * 512],
                                    start=(j == 0), stop=(j == JT - 1))
                    for pair in range(B):
                        nc.vector.tensor_copy(
                            out=oT[:, pair * N + ih * 1024:pair * N + (ih + 1) * 1024],
                            in_=oT_ps[pair])

                # ================= Phase 3: out-projection =================
                for c in range(NCHUNK):
                    y_ps = psA.tile([128, DIM], F32, tag="sc")
                    nc.tensor.matmul(y_ps, oT[0:D, c * 128:(c + 1) * 128], wout_sb,
                                     start=True, stop=True)
                    y_sb = work.tile([128, DIM], F32, tag="ysb")
                    nc.vector.tensor_copy(out=y_sb, in_=y_ps)
                    nc.sync.dma_start(out=y_d.ap()[c * 128:(c + 1) * 128, :], in_=y_sb)

                nc.gpsimd.dma_start(out=s_d.ap(), in_=oT[D:D + 1, :])

                if dump:
                    dbg_qT = nc.dram_tensor("dbg_qT", [D, T], F32R,
                                            kind="ExternalOutput")
                    dbg_kT = nc.dram_tensor("dbg_kT", [D, T], F32R,
                                            kind="ExternalOutput")
                    dbg_v = nc.dram_tensor("dbg_v", [128, B * JT * (D + 1)], F32R,
                                           kind="ExternalOutput")
                    dbg_oT = nc.dram_tensor("dbg_oT", [D + 1, T], F32R,
                                            kind="ExternalOutput")
                    nc.gpsimd.dma_start(out=dbg_qT.ap(), in_=qT)
                    nc.gpsimd.dma_start(out=dbg_kT.ap(), in_=kT)
                    nc.gpsimd.dma_start(
                        out=dbg_v.ap(),
                        in_=vext.rearrange("p a b c -> p (a b c)"))
                    nc.gpsimd.dma_start(out=dbg_oT.ap(), in_=oT)

    nc.compile()
    return nc


def _host_bias(pos):
    # exp(bias), computed with jax ops mirroring the reference verbatim —
    # jnp's % has backend-specific semantics that plain numpy does not match.
    import jax.numpy as jnp
    p = jnp.asarray(pos, jnp.float32)
    dist = jnp.abs(p[:, None, :] - p[None, :, :])
    dist = (dist + 0.5) % 1.0 - 0.5
    edist = jnp.sum(dist ** 2, axis=-1)
    return np.asarray(jnp.exp(-edist), dtype=np.float32)


def kernel(x, pos, W_in, b_in, qn_w, qn_b, kn_w, kn_b, W_out, b_out):
    global _COMPILED
    x = np.ascontiguousarray(np.asarray(x, dtype=np.float32)).reshape(T, DIM)
    pos = np.asarray(pos, dtype=np.float32)
    W_in = np.asarray(W_in, dtype=np.float32)
    b_in = np.asarray(b_in, dtype=np.float32)
    W_out = np.asarray(W_out, dtype=np.float32)
    b_out = np.asarray(b_out, dtype=np.float32)
    # q-side LN affine absorbs the 1/sqrt(d) score scaling
    lnp = np.stack([np.asarray(qn_w, np.float32) / np.float32(np.sqrt(D)),
                    np.asarray(qn_b, np.float32) / np.float32(np.sqrt(D)),
                    np.asarray(kn_w, np.float32), np.asarray(kn_b, np.float32)],
                   axis=1)  # [64, 4]
    bias = _host_bias(pos)

    if _COMPILED is None:
        _COMPILED = _build()
    nc = _COMPILED

    in_maps = []
    for c in range(8):
        cols = np.concatenate([
            W_in[:, 0 * DIM + c * D:0 * DIM + (c + 1) * D],
            W_in[:, 1 * DIM + c * D:1 * DIM + (c + 1) * D],
            W_in[:, 2 * DIM + c * D:2 * DIM + (c + 1) * D]], axis=1)
        bcols = np.concatenate([
            b_in[0 * DIM + c * D:0 * DIM + (c + 1) * D],
            b_in[1 * DIM + c * D:1 * DIM + (c + 1) * D],
            b_in[2 * DIM + c * D:2 * DIM + (c + 1) * D]])[None, :]
        in_maps.append({
            "x": x,
            "wqkv": np.ascontiguousarray(cols),
            "bqkv": np.ascontiguousarray(bcols),
            "lnp": lnp,
            "wout": np.ascontiguousarray(W_out[c * D:(c + 1) * D, :]),
            "bias": bias,
        })

    res = bass_utils.run_bass_kernel_spmd(nc, in_maps, core_ids=list(range(8)),
                                          trace=False)

    acc = np.zeros((T, DIM), dtype=np.float64)
    for c in range(8):
        yc = res.results[c]["y"].astype(np.float64)
        sc = res.results[c]["s"].reshape(T).astype(np.float64)
        acc += yc / sc[:, None]
    out = acc + b_out.astype(np.float64)
    return out.reshape(B, N, DIM).astype(np.float32)


# revision 41
# speedup vs baseline: 1.3753x; 1.3753x over previous
"""Trainium2 Bass kernel for CombinedAttention+LN (b=2, n=2048, dim=512, h=8).

Sharding: head-parallel across 8 cores (core c owns head c for both batches).
Per core: QKV projection for its head (tensor-parallel W_in column slice),
q/k layernorm, attention with host-precomputed positional bias, out-proj
partial product (W_out row slice). Host sums the 8 partials (after per-row
softmax-denominator scaling, which commutes with the out-projection) and
adds b_out.
"""

import sys

sys.path.insert(0, "/opt/trn_rl_repo")

import numpy as np

import concourse.bass as bass
import concourse.mybir as mybir
import concourse.tile as tile
from concourse import bacc
from concourse import bass_utils
from concourse.masks import make_identity

F32 = mybir.dt.float32
F32R = mybir.dt.float32r
AF = mybir.ActivationFunctionType
OP = mybir.AluOpType

B, N, DIM, H = 2, 2048, 512, 8
D = DIM // H          # 64
T = B * N             # 4096 tokens total (both batches)
NCHUNK = T // 128     # 32 token chunks of 128
JT = N // 128         # 16 key tiles per pair
EPS = 1e-5

_COMPILED = None


def _build(dump=False):
    nc = bacc.Bacc("TRN2", target_bir_lowering=False, debug=False, num_devices=8)

    x_d = nc.dram_tensor("x", [T, DIM], F32, kind="ExternalInput")
    wqkv_d = nc.dram_tensor("wqkv", [DIM, 256], F32R, kind="ExternalInput")
    bqkv_d = nc.dram_tensor("bqkv", [1, 256], F32R, kind="ExternalInput")
    lnp_d = nc.dram_tensor("lnp", [D, 4], F32, kind="ExternalInput")
    wout_d = nc.dram_tensor("wout", [D, DIM], F32R, kind="ExternalInput")
    bias_d = nc.dram_tensor("bias", [N, N], F32, kind="ExternalInput")
    y_d = nc.dram_tensor("y", [T, DIM], F32, kind="ExternalOutput")
    s_d = nc.dram_tensor("s", [1, T], F32R, kind="ExternalOutput")

    with tile.TileContext(nc) as tc:
        with tc.tile_pool(name="const", bufs=1) as const, \
             tc.tile_pool(name="big", bufs=1) as big, \
             tc.tile_pool(name="work", bufs=5) as work, \
             tc.tile_pool(name="ln", bufs=8) as lnp_pool, \
             tc.tile_pool(name="psA", bufs=2, space="PSUM") as psA:

            # ---- constants ----
            wqkv_sb = const.tile([128, 4, 256], F32R, tag="wqkv")
            nc.sync.dma_start(
                out=wqkv_sb, in_=wqkv_d.ap().rearrange("(s p) f -> p s f", p=128)
            )
            bqkv_sb = const.tile([1, 256], F32R, tag="bqkv")
            nc.sync.dma_start(out=bqkv_sb, in_=bqkv_d.ap())
            lnp_sb = const.tile([D, 4], F32, tag="lnp")
            nc.sync.dma_start(out=lnp_sb, in_=lnp_d.ap())
            wout_sb = const.tile([D, DIM], F32R, tag="wout")
            nc.sync.dma_start(out=wout_sb, in_=wout_d.ap())
            ident = const.tile([128, 128], F32, tag="ident")
            make_identity(nc, ident)
            ident_r = const.tile([128, 128], F32R, tag="identr")
            make_identity(nc, ident_r.bitcast(F32))
            ones1 = const.tile([1, 128], F32R, tag="ones1")
            nc.vector.memset(ones1.bitcast(F32), 1.0)
            eps_t = const.tile([128, 1], F32, tag="eps")
            nc.vector.memset(eps_t, EPS)

            # ---- persistent buffers ----
            qT = big.tile([D, T], F32R, tag="qT")        # [64, 4096]
            kT = big.tile([D, T], F32R, tag="kT")
            vext = big.tile([128, B, JT, D + 1], F32R, tag="vext")
            oT = big.tile([D + 1, T], F32R, tag="oT")     # rows 0..63 attnT, row 64 sums

            # ================= Phase 1: QKV + LN + transposes =================
            with tc.tile_pool(name="psB", bufs=2, space="PSUM") as psB:
                for jj in range(JT):
                  for pair in range(B):
                    c = pair * JT + jj
                    j = jj
                    xc = work.tile([128, DIM], F32, tag="xc")
                    nc.sync.dma_start(out=xc, in_=x_d.ap()[c * 128:(c + 1) * 128, :])

                    # 4 transposes into one PSUM bank, one copy out (cast f32r)
                    xt_ps = psB.tile([128, 512], F32, tag="tr")
                    for s4 in range(4):
                        nc.tensor.transpose(xt_ps[:, s4 * 128:(s4 + 1) * 128],
                                            xc[:, s4 * 128:(s4 + 1) * 128], ident)
                    xTc = work.tile([128, 4, 128], F32R, tag="xTc")
                    if c % 2 == 0:
                        nc.scalar.copy(out=xTc.rearrange("p a b -> p (a b)"), in_=xt_ps)
                    else:
                        nc.vector.tensor_copy(out=xTc.rearrange("p a b -> p (a b)"),
                                              in_=xt_ps)

                    qkv_ps = psA.tile([128, 256], F32, tag="sc")
                    for s4 in range(4):
                        nc.tensor.matmul(qkv_ps, xTc[:, s4, :], wqkv_sb[:, s4, :],
                                         start=(s4 == 0), stop=False)
                    nc.tensor.matmul(qkv_ps, ones1, bqkv_sb, start=False, stop=True)

                    # v straight from PSUM (+ ones col)
                    if c % 2 == 0:
                        nc.vector.tensor_copy(out=vext[:, pair, j, 0:D],
                                              in_=qkv_ps[:, 2 * D:3 * D])
                    else:
                        nc.scalar.copy(out=vext[:, pair, j, 0:D],
                                       in_=qkv_ps[:, 2 * D:3 * D])
                    nc.gpsimd.memset(vext[:, pair, j, D:D + 1].bitcast(F32), 1.0)

                    # LN stats for q and k (read PSUM directly)
                    st = lnp_pool.tile([128, 2, 6], F32, tag="st")
                    nc.vector.bn_stats(out=st[:, 0, :], in_=qkv_ps[:, 0:D])
                    nc.vector.bn_stats(out=st[:, 1, :], in_=qkv_ps[:, D:2 * D])
                    mv = lnp_pool.tile([128, 2, 2], F32, tag="mv")
                    nc.vector.bn_aggr(out=mv[:, 0, :], in_=st[:, 0, :])
                    nc.vector.bn_aggr(out=mv[:, 1, :], in_=st[:, 1, :])
                    # rstd = 1/sqrt(var + eps) for both q,k in one go (strided var view)
                    sd = lnp_pool.tile([128, 2], F32, tag="sd")
                    nc.scalar.activation(out=sd, in_=mv[:, :, 1], func=AF.Sqrt,
                                         bias=eps_t, scale=1.0)
                    r2 = lnp_pool.tile([128, 2], F32, tag="r2")
                    nc.vector.reciprocal(out=r2, in_=sd)

                    # apply LN: (x - mu) * r, then transpose, then w*x+b into qT/kT
                    qs = work.tile([128, 2, D], F32, tag="qs")
                    for qi in range(2):
                        nc.vector.tensor_scalar(
                            out=qs[:, qi, :], in0=qkv_ps[:, qi * D:(qi + 1) * D],
                            scalar1=mv[:, qi, 0:1], scalar2=r2[:, qi:qi + 1],
                            op0=OP.subtract, op1=OP.mult)
                    t_ps = psB.tile([D, 256], F32, tag="tr2")
                    for qi in range(2):
                        nc.tensor.transpose(t_ps[:, qi * 128:(qi + 1) * 128],
                                            qs[:, qi, :], ident)
                    # q half on ACT (scale/bias APs), k half on DVE
                    nc.scalar.activation(
                        out=qT[:, c * 128:(c + 1) * 128], in_=t_ps[:, 0:128],
                        func=AF.Identity, scale=lnp_sb[:, 0:1], bias=lnp_sb[:, 1:2])
                    nc.vector.tensor_scalar(
                        out=kT[:, c * 128:(c + 1) * 128], in0=t_ps[:, 128:256],
                        scalar1=lnp_sb[:, 2:3], scalar2=lnp_sb[:, 3:4],
                        op0=OP.mult, op1=OP.add)

            # ================= Phase 2: attention =================
            with tc.tile_pool(name="psC", bufs=2, space="PSUM") as psC, \
                 tc.tile_pool(name="bpool", bufs=4) as bpool, \
                 tc.tile_pool(name="epool", bufs=6) as epool:
                for ih in range(2):
                    oT_ps = [psC.tile([D + 1, 1024], F32, tag="oT",
                                      name=f"oT_ps_{ih}_{p_}") for p_ in range(B)]
                    pending = None  # (j, [e_sb per pair]) waiting for EV
                    for j in range(JT):
                        bt = bpool.tile([128, 1024], F32, tag="bt")
                        nc.scalar.dma_start(
                            out=bt,
                            in_=bias_d.ap()[j * 128:(j + 1) * 128,
                                            ih * 1024:(ih + 1) * 1024])
                        es = []
                        for pair in range(B):
                            s_ps = psA.tile([128, 1024], F32, tag="sc")
                            for nh in range(2):
                                nc.tensor.matmul(
                                    s_ps[:, nh * 512:(nh + 1) * 512],
                                    kT[:, pair * N + j * 128:pair * N + (j + 1) * 128],
                                    qT[:, pair * N + ih * 1024 + nh * 512:
                                       pair * N + ih * 1024 + (nh + 1) * 512],
                                    start=True, stop=True)
                            e0_sb = epool.tile([128, 1024], F32, tag="e0",
                                               name=f"e0_{ih}_{j}_{pair}")
                            nc.scalar.activation(out=e0_sb, in_=s_ps, func=AF.Exp)
                            e_sb = epool.tile([128, 1024], F32R, tag="e",
                                              name=f"e_{ih}_{j}_{pair}")
                            eng = nc.vector if (j * B + pair) % 5 < 3 else nc.gpsimd
                            eng.tensor_tensor(out=e_sb, in0=e0_sb, in1=bt, op=OP.mult)
                            es.append(e_sb)
                        if pending is not None:
                            pj, pes = pending
                            for pair in range(B):
                                for nh in range(2):
                                    nc.tensor.matmul(
                                        oT_ps[pair][:, nh * 512:(nh + 1) * 512],
                                        vext[:, pair, pj, :],
                                        pes[pair][:, nh * 512:(nh + 1) * 512],
                                        start=(pj == 0), stop=False)
                        pending = (j, es)
                    pj, pes = pending
                    for pair in range(B):
                        for nh in range(2):
                            nc.tensor.matmul(
                                oT_ps[pair][:, nh * 512:(nh + 1) * 512],
                                vext[:, pair, pj, :],
                                pes[pair][:, nh * 512:(nh + 1) * 512],
                                start=False, stop=True)
                    for pair in range(B):
                        nc.vector.tensor_copy(
                            out=oT[:, pair * N + ih * 1024:pair * N + (ih + 1) * 1024],
                            in_=oT_ps[pair])
                        # out-projection for the 8 token chunks just finished
                        for c in range(pair * JT + ih * 8, pair * JT + ih * 8 + 8):
                            y_ps = psA.tile([128, DIM], F32, tag="sc")
                            nc.tensor.matmul(y_ps, oT[0:D, c * 128:(c + 1) * 128],
                                             wout_sb, start=True, stop=True)
                            y_sb = work.tile([128, DIM], F32, tag="ysb")
                            if c % 2 == 0:
                                nc.vector.tensor_copy(out=y_sb, in_=y_ps)
                            else:
                                nc.scalar.copy(out=y_sb, in_=y_ps)
                            nc.sync.dma_start(out=y_d.ap()[c * 128:(c + 1) * 128, :],
                                              in_=y_sb)

                nc.gpsimd.dma_start(out=s_d.ap(), in_=oT[D:D + 1, :])

                if dump:
                    dbg_qT = nc.dram_tensor("dbg_qT", [D, T], F32R,
                                            kind="ExternalOutput")
                    dbg_kT = nc.dram_tensor("dbg_kT", [D, T], F32R,
                                            kind="ExternalOutput")
                    dbg_v = nc.dram_tensor("dbg_v", [128, B * JT * (D + 1)], F32R,
                                           kind="ExternalOutput")
                    dbg_oT = nc.dram_tensor("dbg_oT", [D + 1, T], F32R,
                                            kind="ExternalOutput")
                    nc.gpsimd.dma_start(out=dbg_qT.ap(), in_=qT)
                    nc.gpsimd.dma_start(out=dbg_kT.ap(), in_=kT)
                    nc.gpsimd.dma_start(
                        out=dbg_v.ap(),
                        in_=vext.rearrange("p a b c -> p (a b c)"))
                    nc.gpsimd.dma_start(out=dbg_oT.ap(), in_=oT)

    nc.compile()
    return nc


def _host_bias(pos):
    # exp(-edist), edist computed with jax ops mirroring the reference verbatim —
    # jnp's % has backend-specific semantics that plain numpy does not match.
    import jax.numpy as jnp
    p = jnp.asarray(pos, jnp.float32)
    dist = jnp.abs(p[:, None, :] - p[None, :, :])
    dist = (dist + 0.5) % 1.0 - 0.5
    edist = jnp.sum(dist ** 2, axis=-1)
    return np.asarray(jnp.exp(-edist), dtype=np.float32)


def kernel(x, pos, W_in, b_in, qn_w, qn_b, kn_w, kn_b, W_out, b_out):
    global _COMPILED
    x = np.ascontiguousarray(np.asarray(x, dtype=np.float32)).reshape(T, DIM)
    pos = np.asarray(pos, dtype=np.float32)
    W_in = np.asarray(W_in, dtype=np.float32)
    b_in = np.asarray(b_in, dtype=np.float32)
    W_out = np.asarray(W_out, dtype=np.float32)
    b_out = np.asarray(b_out, dtype=np.float32)
    # q-side LN affine absorbs the 1/sqrt(d) score scaling
    lnp = np.stack([np.asarray(qn_w, np.float32) / np.float32(np.sqrt(D)),
                    np.asarray(qn_b, np.float32) / np.float32(np.sqrt(D)),
                    np.asarray(kn_w, np.float32), np.asarray(kn_b, np.float32)],
                   axis=1)  # [64, 4]
    bias = _host_bias(pos)

    if _COMPILED is None:
        _COMPILED = _build()
    nc = _COMPILED

    in_maps = []
    for c in range(8):
        cols = np.zeros((DIM, 256), dtype=np.float32)
        cols[:, 0:3 * D] = np.concatenate([
            W_in[:, 0 * DIM + c * D:0 * DIM + (c + 1) * D],
            W_in[:, 1 * DIM + c * D:1 * DIM + (c + 1) * D],
            W_in[:, 2 * DIM + c * D:2 * DIM + (c + 1) * D]], axis=1)
        bcols = np.zeros((1, 256), dtype=np.float32)
        bcols[0, 0:3 * D] = np.concatenate([
            b_in[0 * DIM + c * D:0 * DIM + (c + 1) * D],
            b_in[1 * DIM + c * D:1 * DIM + (c + 1) * D],
            b_in[2 * DIM + c * D:2 * DIM + (c + 1) * D]])
        in_maps.append({
            "x": x,
            "wqkv": np.ascontiguousarray(cols),
            "bqkv": np.ascontiguousarray(bcols),
            "lnp": lnp,
            "wout": np.ascontiguousarray(W_out[c * D:(c + 1) * D, :]),
            "bias": bias,
        })

    res = bass_utils.run_bass_kernel_spmd(nc, in_maps, core_ids=list(range(8)),
                                          trace=False)

    acc = np.zeros((T, DIM), dtype=np.float64)
    for c in range(8):
        yc = res.results[c]["y"].astype(np.float64)
        sc = res.results[c]["s"].reshape(T).astype(np.float64)
        acc += yc / sc[:, None]
    out = acc + b_out.astype(np.float64)
    return out.reshape(B, N, DIM).astype(np.float32)
